# revision 27
# baseline (speedup 1.0000x reference)
"""Trainium2 Bass kernel for nn_AE_RNN (2-layer GRU AE, scan over T, scalar loss).

Strategy: data-parallel over batch across 8 NeuronCores (128 rows/core), no
collectives (host sums the 8 partial losses).  On each core, feature-major
layout [feature, t*128 + b]:
  - bulk phi_u MLP precompute per time-chunk,
  - serial GRU scan (gates in PSUM, sigmoid/tanh on ScalarE, fused DVE ops),
  - measurement branch (dynn -> x_mean/logvar -> phi_x -> menn + C) pipelined
    4 timesteps at a time behind the scan,
  - loss as sum(yhat^2) - 2*sum(yhat*y) on-chip + sum(y^2) on host.
"""

import numpy as np
import ml_dtypes

import concourse.bass as bass
import concourse.tile as tile
from concourse.tile import add_dep_helper
from concourse import bacc, mybir
from concourse.bass_utils import run_bass_kernel_spmd

B, T, UD, YD, ZD, HD, L = 1024, 1024, 16, 16, 16, 128, 2
NCORES = 8
BL = B // NCORES  # 128

BF = mybir.dt.bfloat16
F32 = mybir.dt.float32
AF = mybir.ActivationFunctionType
OP = mybir.AluOpType
BF_NP = ml_dtypes.bfloat16


def build(T_total=T, Tc=64, reps=1):
    """Build the per-core Bass graph.  Returns (nc, meta).

    reps>1 re-emits the whole pipeline (for marginal-time measurement);
    results are only meaningful for reps=1."""
    assert T_total % Tc == 0 and Tc % 4 == 0
    NCH = T_total // Tc          # number of time chunks
    NG = Tc // 4                 # measurement / bulk groups per chunk
    NGT = T_total // 4           # total groups
    NQ = NGT                     # total quads (same thing)
    GW = 4 * BL                  # group width in columns (512)

    nc = bacc.Bacc("TRN2", target_bir_lowering=False, debug=False)

    def param(name, shape, dt=BF):
        return nc.declare_dram_parameter(name, list(shape), dt, isOutput=False)

    u_p = param("u", [UD, T_total * BL])
    y_p = param("y", [YD, T_total * BL])
    h0_p = param("h0f", [HD, 2 * BL])
    w_shapes = dict(
        wih0T=(HD, 3 * HD), whh0T=(HD, 3 * HD),
        wih1T=(HD, 3 * HD), whh1T=(HD, 3 * HD),
        pw0T=(UD, HD), b3=(3, HD), ind3=(3, 3 * HD),
        dw0aT=(HD, HD), dw0bT=(HD, HD),
        fwT=(HD, HD),          # host-fused dynn1 -> phi_x hidden
        gT=(HD, HD),           # host-fused phi_x out -> menn hidden
        mw1T=(HD, YD), cxwT=(HD, YD),   # cxwT: host-fused dh -> C@x_mean
        ident=(HD, HD),
    )
    b_shapes = dict(
        pb0=(HD, 1), db0=(HD, 1),
        fb=(HD, 1), gb=(HD, 1),
        mb1=(YD, 1),
    )
    w_params = {n: param(n, s) for n, s in w_shapes.items()}
    b_params = {n: param(n, s, F32) for n, s in b_shapes.items()}
    out_p = nc.declare_dram_parameter("out", [YD, 2 * NGT], F32, isOutput=True)

    with tile.TileContext(nc) as tc:
        with (
            tc.tile_pool(name="const", bufs=1) as const,
            tc.tile_pool(name="big", bufs=1) as big,
            tc.tile_pool(name="io", bufs=2) as io,
            tc.tile_pool(name="scan", bufs=3) as scan,
            tc.tile_pool(name="meas", bufs=2) as meas,
            tc.tile_pool(name="psg_p", bufs=3, space="PSUM") as psg_p,
            tc.tile_pool(name="psm_p", bufs=3, space="PSUM") as psm_p,
            tc.tile_pool(name="psb_p", bufs=2, space="PSUM") as psb_p,
        ):
            mm = nc.tensor.matmul
            act = nc.scalar.activation
            dma = nc.sync.dma_start

            # ---- constants into SBUF ----
            W = {}
            for n, s in w_shapes.items():
                W[n] = const.tile(list(s), BF, tag=n, name=n)
                dma(W[n][:], w_params[n][:])
            BI = {}
            for n, s in b_shapes.items():
                BI[n] = const.tile(list(s), F32, tag=n, name="b_" + n)
                dma(BI[n][:], b_params[n][:])

            # ---- persistent state ----
            h0ring = big.tile([HD, 5 * BL], BF, tag="h0ring", name="h0ring")
            h_hist = big.tile([HD, (Tc + 1) * BL], BF, tag="h_hist", name="h_hist")
            phis = [big.tile([HD, Tc * BL], BF, tag=f"phi{i}", name=f"phi{i}") for i in range(2)]
            sq_acc = big.tile([YD, NGT], F32, tag="sq_acc", name="sq_acc")
            xy_acc = big.tile([YD, NGT], F32, tag="xy_acc", name="xy_acc")

            dma(h0ring[:, 0:BL], h0_p[:, 0:BL])
            dma(h_hist[:, 0:BL], h0_p[:, BL:2 * BL])

            def chunk_cols(ci):
                return slice(ci * Tc * BL, (ci + 1) * Tc * BL)

            def load_uy(ci):
                uc = io.tile([UD, Tc * BL], BF, tag="u", name="u")
                dma(uc[:], u_p[:, chunk_cols(ci)])
                yc = io.tile([YD, Tc * BL], BF, tag="y", name="y")
                dma(yc[:], y_p[:, chunk_cols(ci)])
                return uc, yc

            def gate_bias(psg):
                # rank-3 bias matmul: psg[:, 0:3BL] = b3 (gate-wise) via the
                # 0/1 indicator moving operand; opens all three accum ranges
                mm(psg[:, 0:3 * BL], W["b3"][:], W["ind3"][:],
                   start=True, stop=False)

            def gate_mms_x(psg, wi, x_ap, first, close_n=False):
                # input-side matmuls (r, z accumulate-partners, n separate)
                mm(psg[:, 0:BL], W[wi][:, 0:HD], x_ap,
                   start=first, stop=False)
                mm(psg[:, BL:2 * BL], W[wi][:, HD:2 * HD], x_ap,
                   start=first, stop=False)
                mm(psg[:, 2 * BL:3 * BL], W[wi][:, 2 * HD:3 * HD], x_ap,
                   start=first, stop=close_n)

            def gate_mms_h(psg, wh, h_ap, r_first):
                # hidden-side matmuls (r, z close their accum ranges; hn
                # has its own range; gin range is closed by the I@t matmul)
                if r_first:
                    # L0 critical path: close r first (sigmoid_r can start),
                    # then hn (t-product), z last.
                    mm(psg[:, 0:BL], W[wh][:, 0:HD], h_ap,
                       start=False, stop=True)
                    last = mm(psg[:, 3 * BL:4 * BL], W[wh][:, 2 * HD:3 * HD],
                              h_ap, start=True, stop=True)
                    mm(psg[:, BL:2 * BL], W[wh][:, HD:2 * HD], h_ap,
                       start=False, stop=True)
                else:
                    mm(psg[:, 0:BL], W[wh][:, 0:HD], h_ap,
                       start=False, stop=True)
                    mm(psg[:, BL:2 * BL], W[wh][:, HD:2 * HD], h_ap,
                       start=False, stop=True)
                    last = mm(psg[:, 3 * BL:4 * BL], W[wh][:, 2 * HD:3 * HD],
                              h_ap, start=True, stop=True)
                return last

            def gate_r(psg, split):
                # r|z sigmoid; split=True issues r alone first so the
                # t-product can start ~100ns earlier (L0 critical path)
                rT = scan.tile([HD, 2 * BL], BF, tag="rT", name="rT", bufs=4)
                if split:
                    act(rT[:, 0:BL], psg[:, 0:BL], AF.Sigmoid)
                    act(rT[:, BL:2 * BL], psg[:, BL:2 * BL], AF.Sigmoid)
                else:
                    act(rT[:], psg[:, 0:2 * BL], AF.Sigmoid)
                return rT

            def gate_z(psg):
                return None

            def gate_zb(z_ap):
                # zbar = 1 - z on GpSimd (reads the bf16 z, SBUF only)
                zb = scan.tile([HD, BL], BF, tag="zb", name="zb", bufs=4)
                nc.gpsimd.tensor_scalar(zb[:], z_ap, -1.0, 1.0,
                                        op0=OP.mult, op1=OP.add)
                return zb

            def gate_tm(psg, rT):
                t0 = scan.tile([HD, BL], BF, tag="t0", name="t0", bufs=4)
                nc.vector.tensor_mul(t0[:], rT[:, 0:BL], psg[:, 3 * BL:4 * BL])
                # m = gin + t via PE accumulate (closes the gin accum group)
                imm = mm(psg[:, 2 * BL:3 * BL], W["ident"][:], t0[:],
                         start=False, stop=True)
                anchors["PE"] = imm
                return psg[:, 2 * BL:3 * BL]

            def gate_tm_dve(psg, rT):
                # L1 variant: keep the m-add OFF the PE queue so the next
                # step's L0 matmuls aren't stuck behind it (PE is in-order).
                t1 = scan.tile([HD, BL], BF, tag="t1", name="t1", bufs=4)
                nc.vector.tensor_mul(t1[:], rT[:, 0:BL], psg[:, 3 * BL:4 * BL])
                m1 = scan.tile([HD, BL], F32, tag="m1", name="m1", bufs=4)
                nc.vector.tensor_add(m1[:], psg[:, 2 * BL:3 * BL], t1[:])
                return m1

            def gate_a(z_ap, h_ap):
                a = scan.tile([HD, BL], BF, tag="ga", name="ga", bufs=6)
                nc.gpsimd.tensor_mul(a[:], z_ap, h_ap)
                return a

            def gru_cell(layer, psg, x_ap, h_prev_ap, h_dst_ap, use_bias):
                if use_bias:
                    gate_bias(psg)
                gate_mms_x(psg, "wih%dT" % layer, x_ap, not use_bias,
                           close_n=False)
                gate_mms_h(psg, "whh%dT" % layer, h_prev_ap,
                           r_first=False)
                rT = gate_r(psg, split=False)
                z = rT[:, BL:2 * BL]
                zb = gate_zb(z)
                a = gate_a(z, h_prev_ap)
                if layer == 0:
                    m = gate_tm(psg, rT)
                else:
                    m = gate_tm_dve(psg, rT)
                anchors["ACT"] = act(h_dst_ap, m[:], AF.Tanh)
                b = scan.tile([HD, BL], BF, tag="gb", name="gb", bufs=6)
                nc.vector.tensor_mul(b[:], zb[:], h_dst_ap)
                if layer == 0:
                    anchors["DVE"] = nc.vector.tensor_add(h_dst_ap, a[:], b[:])
                else:
                    # L1's tail add on GpSimd: keeps the in-order DVE queue
                    # clear of late L1 ops that would block the next phase's
                    # L0 t-product.
                    nc.gpsimd.tensor_add(h_dst_ap, a[:], b[:])

            def emit_scan_step(u, phic_of):
                """L0 of step u, then L1 of step u-2.  L1 lags TWO phases so
                its tail ops (which sit in the in-order ACT/DVE queues ahead
                of the next phase's L0 ops) retire well before they can block
                the L0 recurrence."""
                do0 = u < T_total
                do1 = 2 <= u <= T_total + 1
                if do0:
                    ci0, tl0 = divmod(u, Tc)
                    # true 5-slot ring: h0(t) lives at slot (t+1)%5 — no
                    # wrap-around copy needed (WAR distance >= 2 phases)
                    kp = u % 5
                    kd = (u + 1) % 5
                    psg0 = psg_p.tile([HD, 4 * BL], F32, tag="psg", name="psg")
                    gru_cell(0, psg0,
                             phic_of(ci0)[:, tl0 * BL:(tl0 + 1) * BL],
                             h0ring[:, kp * BL:(kp + 1) * BL],
                             h0ring[:, kd * BL:(kd + 1) * BL],
                             use_bias=True)
                if do1:
                    t1 = u - 2
                    ci1, tl1 = divmod(t1, Tc)
                    kx = (t1 + 1) % 5      # slot of h0(t1)
                    psg1 = psg_p.tile([HD, 4 * BL], F32, tag="psg", name="psg")
                    gru_cell(1, psg1,
                             h0ring[:, kx * BL:(kx + 1) * BL],
                             h_hist[:, tl1 * BL:(tl1 + 1) * BL],
                             h_hist[:, (tl1 + 1) * BL:(tl1 + 2) * BL],
                             use_bias=False)
                    if tl1 == Tc - 1:
                        # h1 carry into next chunk: slot Tc -> slot 0
                        nc.gpsimd.tensor_copy(h_hist[:, 0:BL],
                                              h_hist[:, Tc * BL:(Tc + 1) * BL])

            # Measurement chain, software-pipelined: stage s of quad-group
            # gg executes at quad gg+s so every PE matmul's inputs are ready
            # before PE reaches it (PE streams are in-order; a waiting
            # matmul would block later scan matmuls).
            meas_state = {}
            N_MEAS_STAGES = 4
            HW2 = GW // 2

            anchors = {"ACT": None, "DVE": None, "PE": None}

            def anchored(eng, op):
                a = anchors[eng]
                if a is not None:
                    add_dep_helper(op.ins, a.ins, sync=False,
                                   reason="fill inter-step gap only")
                return op

            def ts_split(dst, srcp, bias, relu, base=0):
                # anchored after the scan chain tail, so these only occupy
                # the inter-step gap in the DVE stream
                d = dst[:, base:base + GW]
                if relu:
                    anchored("DVE", nc.vector.tensor_scalar(
                        d, srcp[:], bias, 0.0, op0=OP.add, op1=OP.max))
                else:
                    anchored("DVE", nc.vector.tensor_scalar(
                        d, srcp[:], bias, None, op0=OP.add))

            def meas_stage(s, gg, phic_of, yc_of):
                ci, g = divmod(gg, NG)
                cs = slice(g * GW, (g + 1) * GW)
                st = meas_state.setdefault(gg, {})
                if s == 0:
                    psA = psm_p.tile([HD, GW], F32, tag="pm", name="pm")
                    anchored("PE", mm(psA[:], W["dw0aT"][:], phic_of(ci)[:, cs], start=True, stop=False))
                    mm(psA[:], W["dw0bT"][:], h_hist[:, cs], start=False, stop=True)
                    st["dh"] = meas.tile([HD, GW], BF, tag="dh", name="dh", bufs=4)
                    anchored("ACT", act(st["dh"][:], psA[:], AF.Relu,
                                        bias=BI["db0"][:]))
                elif s == 1:
                    psD = psm_p.tile([HD, GW], F32, tag="pm", name="pm")
                    anchored("PE", mm(psD[:], W["fwT"][:], st["dh"][:], start=True, stop=True))
                    st["ph"] = meas.tile([HD, GW], BF, tag="ph", name="ph", bufs=3)
                    anchored("ACT", act(st["ph"][:], psD[:], AF.Relu,
                                        bias=BI["fb"][:]))
                elif s == 2:
                    psF = psm_p.tile([HD, GW], F32, tag="pm", name="pm")
                    anchored("PE", mm(psF[:], W["gT"][:], st.pop("ph")[:], start=True, stop=True))
                    st["mh"] = meas.tile([HD, GW], BF, tag="mh", name="mh", bufs=3)
                    anchored("ACT", act(st["mh"][:], psF[:], AF.Relu,
                                        bias=BI["gb"][:]))
                elif s == 3:
                    psY = psm_p.tile([YD, GW], F32, tag="pm", name="pm")
                    anchored("PE", mm(psY[:], W["mw1T"][:], st.pop("mh")[:], start=True, stop=False))
                    mm(psY[:], W["cxwT"][:], st.pop("dh")[:], start=False, stop=True)
                    jA = meas.tile([YD, GW], F32, tag="jA", name="jA", bufs=3)
                    anchored("ACT", act(jA[:], psY[:], AF.Square,
                                        bias=BI["mb1"][:],
                                        accum_out=sq_acc[:, gg:gg + 1]))
                    jB = meas.tile([YD, GW], F32, tag="jB", name="jB", bufs=3)
                    anchored("DVE", nc.vector.affine_mul_reduce(
                        jB[:], xy_acc[:, gg:gg + 1],
                        psY[:], yc_of(ci)[:, cs], 1.0, BI["mb1"][:]))
                    del meas_state[gg]

            # Bulk: hid = relu(pw0 @ u + pb0).  phi_u's second linear layer
            # is host-fused into wih0/dw0a, so `hid` IS the scan/meas input.
            def bulk_stage(s, bb, uc_of, phid_of):
                ci, g = divmod(bb, NG)
                cs = slice(g * GW, (g + 1) * GW)
                ps1 = psb_p.tile([HD, GW], F32, tag="pb", name="pb")
                anchored("PE", mm(ps1[:], W["pw0T"][:], uc_of(ci)[:, cs], start=True, stop=True))
                ts_split(phid_of(ci), ps1, BI["pb0"][:], True, base=g * GW)

            # ---- main schedule ----
            for _rep in range(reps):
                uy = {0: load_uy(0)}
                phic_of = lambda ci: phis[ci % 2]
                uc_of = lambda ci: uy[ci][0]
                yc_of = lambda ci: uy[ci][1]
                # prologue: chunk-0 hid
                for g in range(NG):
                    bulk_stage(0, g, uc_of, phic_of)
                if NCH > 1:
                    uy[1] = load_uy(1)
                for u in range(T_total + 10):
                    emit_scan_step(u, phic_of)
                    # measurement: stage s of group gg at step 4*gg+5+s
                    # (h1(t) lands at phase t+2 now that L1 lags two phases)
                    for s in range(N_MEAS_STAGES):
                        r = u - 5 - s
                        if r >= 0 and r % 4 == 0:
                            gg = r // 4
                            if gg < NQ:
                                meas_stage(s, gg, phic_of, yc_of)
                    # bulk for the next chunk: one stage per quad, placed on
                    # the u%4==2 phases (dh on ==1, jA/jB on ==0, mh on ==3;
                    # ==2 has no other DVE work)
                    r = u - 2
                    if r >= 0 and r % 4 == 0:
                        bb = r // 4 + NG
                        if NG <= bb < NQ:
                            bulk_stage(0, bb, uc_of, phic_of)
                    # io prefetch / release bookkeeping at chunk boundaries
                    if u < T_total and u % Tc == Tc - 1:
                        ci = u // Tc
                        if ci + 2 < NCH:
                            uy[ci + 2] = load_uy(ci + 2)
                        uy.pop(ci - 1, None)
            dma(out_p[:, 0:NGT], sq_acc[:])
            dma(out_p[:, NGT:2 * NGT], xy_acc[:])

    nc.compile()
    meta = dict(T_total=T_total, Tc=Tc, NGT=NGT)
    return nc, meta


def prep_inputs(inputs, T_total=T):
    """Host-side shard + relayout.  Returns (in_maps, y_sq_sum)."""
    u = np.asarray(inputs["u"], np.float32)
    y = np.asarray(inputs["y"], np.float32)
    h0 = np.asarray(inputs["h0"], np.float32)
    if T_total != T:
        u = u[:, :, :T_total]
        y = y[:, :, :T_total]

    g = lambda n: np.asarray(inputs[n], np.float64)
    wih, whh = g("gru_wih"), g("gru_whh")
    # host-fused linear-linear boundaries of the measurement branch:
    #   xmv = Xc @ (W1 dh + b1) + xb  ->  XW1 dh + xb2
    #   ph  = relu(Px0 xmv' + pb0)    where xmv' is the same affine of dh
    #   mh  = relu(Mw0 (Px1 ph + pb1) + mb0) -> G ph + gb
    Xc = np.concatenate([g("x_mean_w"), g("x_logvar_w")], 0)   # (32,128)
    xb = np.concatenate([g("x_mean_b"), g("x_logvar_b")], 0)
    XW1 = Xc @ g("dynn_w1")
    xb2 = Xc @ g("dynn_b1") + xb
    FW = g("phi_x_w0") @ XW1
    fb = g("phi_x_w0") @ xb2 + g("phi_x_b0")
    G = g("menn_w0") @ g("phi_x_w1")
    gb = g("menn_w0") @ g("phi_x_b1") + g("menn_b0")
    # physics term C @ x_mean host-fused straight onto dh:
    #   C @ x_mean = (C @ XW1[:ZD]) @ dh + C @ xb2[:ZD]
    CXW = g("C") @ XW1[:ZD]                  # (YD, HD)
    mb2 = g("menn_b1") + g("C") @ xb2[:ZD]   # (YD,)
    # fuse phi_u's second linear layer into all its (linear) consumers:
    #   wih0 @ phi_u = (wih0 @ W1) @ hid + wih0 @ b1
    #   dynn_a @ phi_u = (dynn_a @ W1) @ hid + dynn_a @ b1
    W1, b1 = g("phi_u_w1"), g("phi_u_b1")
    gru_b = wih[0] @ b1                       # (384,) layer-0 gate bias
    ind3 = np.zeros((3, 3 * HD))
    for k in range(3):
        ind3[k, k * HD:(k + 1) * HD] = 1.0
    shared = {
        "wih0T": (wih[0] @ W1).T, "whh0T": whh[0].T,
        "wih1T": wih[1].T, "whh1T": whh[1].T,
        "pw0T": g("phi_u_w0").T,
        "b3": gru_b.reshape(3, HD), "ind3": ind3,
        "dw0aT": (g("dynn_w0")[:, :HD] @ W1).T, "dw0bT": g("dynn_w0")[:, HD:].T,
        "fwT": FW.T, "gT": G.T,
        "mw1T": g("menn_w1").T, "cxwT": CXW.T,
        "ident": np.eye(HD, dtype=np.float64),
    }
    shared = {k: np.ascontiguousarray(v, dtype=BF_NP) for k, v in shared.items()}
    biases = {
        "pb0": g("phi_u_b0"),
        "db0": g("dynn_b0") + g("dynn_w0")[:, :HD] @ b1,
        "fb": fb, "gb": gb,
        "mb1": mb2,
    }
    for k, v in biases.items():
        shared[k] = np.ascontiguousarray(v[:, None], dtype=np.float32)

    in_maps = []
    for c in range(NCORES):
        bs = slice(c * BL, (c + 1) * BL)
        ub = u[bs]                      # (BL, UD, Tt)
        yb = y[bs]
        m = dict(shared)
        m["u"] = np.ascontiguousarray(
            ub.transpose(1, 2, 0).reshape(UD, -1), dtype=BF_NP)
        m["y"] = np.ascontiguousarray(
            yb.transpose(1, 2, 0).reshape(YD, -1), dtype=BF_NP)
        h0b = h0[:, bs, :]              # (L, BL, HD)
        m["h0f"] = np.ascontiguousarray(
            np.concatenate([h0b[0].T, h0b[1].T], 1), dtype=BF_NP)
        in_maps.append(m)

    y_sq = float(np.dot(y.reshape(-1).astype(np.float64),
                        y.reshape(-1).astype(np.float64)))
    return in_maps, y_sq


def reduce_outputs(results, meta, y_sq):
    NGT = meta["NGT"]
    total = 0.0
    for r in results:
        o = np.asarray(r["out"], np.float64)
        total += o[:, :NGT].sum() - 2.0 * o[:, NGT:].sum()
    return np.float32(total + y_sq)


_CACHE = {}


def kernel(**inputs):
    key = ("full", T, 64)
    if key not in _CACHE:
        _CACHE[key] = build(T, 64)
    nc, meta = _CACHE[key]
    in_maps, y_sq = prep_inputs(inputs, T)
    res = run_bass_kernel_spmd(nc, in_maps, core_ids=list(range(NCORES)))
    return reduce_outputs(res.results, meta, y_sq)



# revision 28
# speedup vs baseline: 1.1322x; 1.1322x over previous
"""Trainium2 Bass kernel for nn_AE_RNN (2-layer GRU AE, scan over T, scalar loss).

Strategy: data-parallel over batch across 8 NeuronCores (128 rows/core), no
collectives (host sums the 8 partial losses).  On each core, feature-major
layout [feature, t*128 + b]:
  - bulk phi_u MLP precompute per time-chunk,
  - serial GRU scan (gates in PSUM, sigmoid/tanh on ScalarE, fused DVE ops),
  - measurement branch (dynn -> x_mean/logvar -> phi_x -> menn + C) pipelined
    4 timesteps at a time behind the scan,
  - loss as sum(yhat^2) - 2*sum(yhat*y) on-chip + sum(y^2) on host.
"""

import numpy as np
import ml_dtypes

import concourse.bass as bass
import concourse.tile as tile
from concourse.tile import add_dep_helper
from concourse import bacc, mybir
from concourse.bass_utils import run_bass_kernel_spmd

B, T, UD, YD, ZD, HD, L = 1024, 1024, 16, 16, 16, 128, 2
NCORES = 8
BL = B // NCORES  # 128

BF = mybir.dt.bfloat16
F32 = mybir.dt.float32
AF = mybir.ActivationFunctionType
OP = mybir.AluOpType
BF_NP = ml_dtypes.bfloat16


def build(T_total=T, Tc=64, reps=1):
    """Build the per-core Bass graph.  Returns (nc, meta).

    reps>1 re-emits the whole pipeline (for marginal-time measurement);
    results are only meaningful for reps=1."""
    assert T_total % Tc == 0 and Tc % 4 == 0
    NCH = T_total // Tc          # number of time chunks
    NG = Tc // 4                 # measurement / bulk groups per chunk
    NGT = T_total // 4           # total groups
    NQ = NGT                     # total quads (same thing)
    GW = 4 * BL                  # group width in columns (512)

    nc = bacc.Bacc("TRN2", target_bir_lowering=False, debug=False)

    def param(name, shape, dt=BF):
        return nc.declare_dram_parameter(name, list(shape), dt, isOutput=False)

    u_p = param("u", [UD, T_total * BL])
    y_p = param("y", [YD, T_total * BL])
    h0_p = param("h0f", [HD, 2 * BL])
    w_shapes = dict(
        wih0T=(HD, 3 * HD), whh0T=(HD, 3 * HD),
        wih1T=(HD, 3 * HD), whh1T=(HD, 3 * HD),
        pw0T=(UD, HD), b3=(3, HD), ind3=(3, 3 * HD),
        dw0aT=(HD, HD), dw0bT=(HD, HD),
        fwT=(HD, HD),          # host-fused dynn1 -> phi_x hidden
        gT=(HD, HD),           # host-fused phi_x out -> menn hidden
        mw1T=(HD, YD), cxwT=(HD, YD),   # cxwT: host-fused dh -> C@x_mean
        ident=(HD, HD),
    )
    b_shapes = dict(
        pb0=(HD, 1), db0=(HD, 1),
        fb=(HD, 1), gb=(HD, 1),
        mb1=(YD, 1),
    )
    w_params = {n: param(n, s) for n, s in w_shapes.items()}
    b_params = {n: param(n, s, F32) for n, s in b_shapes.items()}
    out_p = nc.declare_dram_parameter("out", [YD, 2 * NGT], F32, isOutput=True)

    with tile.TileContext(nc) as tc:
        with (
            tc.tile_pool(name="const", bufs=1) as const,
            tc.tile_pool(name="big", bufs=1) as big,
            tc.tile_pool(name="io", bufs=2) as io,
            tc.tile_pool(name="scan", bufs=3) as scan,
            tc.tile_pool(name="meas", bufs=2) as meas,
            tc.tile_pool(name="psg_p", bufs=3, space="PSUM") as psg_p,
            tc.tile_pool(name="psm_p", bufs=3, space="PSUM") as psm_p,
            tc.tile_pool(name="psb_p", bufs=2, space="PSUM") as psb_p,
        ):
            mm = nc.tensor.matmul
            act = nc.scalar.activation
            dma = nc.sync.dma_start

            # ---- constants into SBUF ----
            W = {}
            for n, s in w_shapes.items():
                W[n] = const.tile(list(s), BF, tag=n, name=n)
                dma(W[n][:], w_params[n][:])
            BI = {}
            for n, s in b_shapes.items():
                BI[n] = const.tile(list(s), F32, tag=n, name="b_" + n)
                dma(BI[n][:], b_params[n][:])

            # ---- persistent state ----
            h0ring = big.tile([HD, 5 * BL], BF, tag="h0ring", name="h0ring")
            h_hist = big.tile([HD, (Tc + 1) * BL], BF, tag="h_hist", name="h_hist")
            phis = [big.tile([HD, Tc * BL], BF, tag=f"phi{i}", name=f"phi{i}") for i in range(2)]
            sq_acc = big.tile([YD, NGT], F32, tag="sq_acc", name="sq_acc")
            xy_acc = big.tile([YD, NGT], F32, tag="xy_acc", name="xy_acc")

            dma(h0ring[:, 0:BL], h0_p[:, 0:BL])
            dma(h_hist[:, 0:BL], h0_p[:, BL:2 * BL])

            def chunk_cols(ci):
                return slice(ci * Tc * BL, (ci + 1) * Tc * BL)

            def load_uy(ci):
                uc = io.tile([UD, Tc * BL], BF, tag="u", name="u")
                dma(uc[:], u_p[:, chunk_cols(ci)])
                yc = io.tile([YD, Tc * BL], BF, tag="y", name="y")
                dma(yc[:], y_p[:, chunk_cols(ci)])
                return uc, yc

            def gate_bias(psg):
                # rank-3 bias matmul: psg[:, 0:3BL] = b3 (gate-wise) via the
                # 0/1 indicator moving operand; opens all three accum ranges
                mm(psg[:, 0:3 * BL], W["b3"][:], W["ind3"][:],
                   start=True, stop=False)

            def gate_mms_x(psg, wi, x_ap, first, close_n=False):
                # input-side matmuls (r, z accumulate-partners, n separate)
                mm(psg[:, 0:BL], W[wi][:, 0:HD], x_ap,
                   start=first, stop=False)
                mm(psg[:, BL:2 * BL], W[wi][:, HD:2 * HD], x_ap,
                   start=first, stop=False)
                mm(psg[:, 2 * BL:3 * BL], W[wi][:, 2 * HD:3 * HD], x_ap,
                   start=first, stop=close_n)

            def gate_mms_h(psg, wh, h_ap, r_first):
                # hidden-side matmuls (r, z close their accum ranges; hn
                # has its own range; gin range is closed by the I@t matmul)
                if r_first:
                    # L0 critical path: close r first (sigmoid_r can start),
                    # then hn (t-product), z last.
                    mm(psg[:, 0:BL], W[wh][:, 0:HD], h_ap,
                       start=False, stop=True)
                    last = mm(psg[:, 3 * BL:4 * BL], W[wh][:, 2 * HD:3 * HD],
                              h_ap, start=True, stop=True)
                    mm(psg[:, BL:2 * BL], W[wh][:, HD:2 * HD], h_ap,
                       start=False, stop=True)
                else:
                    mm(psg[:, 0:BL], W[wh][:, 0:HD], h_ap,
                       start=False, stop=True)
                    mm(psg[:, BL:2 * BL], W[wh][:, HD:2 * HD], h_ap,
                       start=False, stop=True)
                    last = mm(psg[:, 3 * BL:4 * BL], W[wh][:, 2 * HD:3 * HD],
                              h_ap, start=True, stop=True)
                return last

            def gate_r(psg, split):
                # r|z sigmoid; split=True issues r alone first so the
                # t-product can start ~100ns earlier (L0 critical path)
                rT = scan.tile([HD, 2 * BL], BF, tag="rT", name="rT", bufs=4)
                if split:
                    act(rT[:, 0:BL], psg[:, 0:BL], AF.Sigmoid)
                    act(rT[:, BL:2 * BL], psg[:, BL:2 * BL], AF.Sigmoid)
                else:
                    act(rT[:], psg[:, 0:2 * BL], AF.Sigmoid)
                return rT

            def gate_z(psg):
                return None

            def gate_zb(z_ap):
                # zbar = 1 - z on GpSimd (reads the bf16 z, SBUF only)
                zb = scan.tile([HD, BL], BF, tag="zb", name="zb", bufs=4)
                nc.gpsimd.tensor_scalar(zb[:], z_ap, -1.0, 1.0,
                                        op0=OP.mult, op1=OP.add)
                return zb

            def gate_tm(psg, rT):
                t0 = scan.tile([HD, BL], BF, tag="t0", name="t0", bufs=4)
                nc.vector.tensor_mul(t0[:], rT[:, 0:BL], psg[:, 3 * BL:4 * BL])
                # m = gin + t via PE accumulate (closes the gin accum group)
                imm = mm(psg[:, 2 * BL:3 * BL], W["ident"][:], t0[:],
                         start=False, stop=True)
                anchors["PE"] = imm
                return psg[:, 2 * BL:3 * BL]

            def gate_tm_dve(psg, rT):
                # L1 variant: keep the m-add OFF the PE queue so the next
                # step's L0 matmuls aren't stuck behind it (PE is in-order).
                t1 = scan.tile([HD, BL], BF, tag="t1", name="t1", bufs=4)
                nc.vector.tensor_mul(t1[:], rT[:, 0:BL], psg[:, 3 * BL:4 * BL])
                m1 = scan.tile([HD, BL], F32, tag="m1", name="m1", bufs=4)
                nc.vector.tensor_add(m1[:], psg[:, 2 * BL:3 * BL], t1[:])
                return m1

            def gate_a(z_ap, h_ap):
                a = scan.tile([HD, BL], BF, tag="ga", name="ga", bufs=6)
                nc.gpsimd.tensor_mul(a[:], z_ap, h_ap)
                return a

            def gru_cell(layer, psg, x_ap, h_prev_ap, h_dst_ap, use_bias):
                if use_bias:
                    gate_bias(psg)
                gate_mms_x(psg, "wih%dT" % layer, x_ap, not use_bias,
                           close_n=False)
                gate_mms_h(psg, "whh%dT" % layer, h_prev_ap,
                           r_first=False)
                rT = gate_r(psg, split=False)
                z = rT[:, BL:2 * BL]
                zb = gate_zb(z)
                a = gate_a(z, h_prev_ap)
                if layer == 0:
                    m = gate_tm(psg, rT)
                else:
                    m = gate_tm_dve(psg, rT)
                anchors["ACT"] = act(h_dst_ap, m[:], AF.Tanh)
                b = scan.tile([HD, BL], BF, tag="gb", name="gb", bufs=6)
                nc.vector.tensor_mul(b[:], zb[:], h_dst_ap)
                if layer == 0:
                    anchors["DVE"] = nc.vector.tensor_add(h_dst_ap, a[:], b[:])
                else:
                    # L1's tail add on GpSimd: keeps the in-order DVE queue
                    # clear of late L1 ops that would block the next phase's
                    # L0 t-product.
                    nc.gpsimd.tensor_add(h_dst_ap, a[:], b[:])

            def emit_scan_step(u, phic_of):
                """L0 of step u, then L1 of step u-2.  L1 lags TWO phases so
                its tail ops (which sit in the in-order ACT/DVE queues ahead
                of the next phase's L0 ops) retire well before they can block
                the L0 recurrence."""
                do0 = u < T_total
                do1 = 2 <= u <= T_total + 1
                if do0:
                    ci0, tl0 = divmod(u, Tc)
                    # true 5-slot ring: h0(t) lives at slot (t+1)%5 — no
                    # wrap-around copy needed (WAR distance >= 2 phases)
                    kp = u % 5
                    kd = (u + 1) % 5
                    psg0 = psg_p.tile([HD, 4 * BL], F32, tag="psg", name="psg")
                    gru_cell(0, psg0,
                             phic_of(ci0)[:, tl0 * BL:(tl0 + 1) * BL],
                             h0ring[:, kp * BL:(kp + 1) * BL],
                             h0ring[:, kd * BL:(kd + 1) * BL],
                             use_bias=True)
                if do1:
                    t1 = u - 2
                    ci1, tl1 = divmod(t1, Tc)
                    kx = (t1 + 1) % 5      # slot of h0(t1)
                    psg1 = psg_p.tile([HD, 4 * BL], F32, tag="psg", name="psg")
                    gru_cell(1, psg1,
                             h0ring[:, kx * BL:(kx + 1) * BL],
                             h_hist[:, tl1 * BL:(tl1 + 1) * BL],
                             h_hist[:, (tl1 + 1) * BL:(tl1 + 2) * BL],
                             use_bias=False)
                    if tl1 == Tc - 1:
                        # h1 carry into next chunk: slot Tc -> slot 0
                        nc.gpsimd.tensor_copy(h_hist[:, 0:BL],
                                              h_hist[:, Tc * BL:(Tc + 1) * BL])

            # Measurement chain, software-pipelined: stage s of quad-group
            # gg executes at quad gg+s so every PE matmul's inputs are ready
            # before PE reaches it (PE streams are in-order; a waiting
            # matmul would block later scan matmuls).
            meas_state = {}
            N_MEAS_STAGES = 4
            HW2 = GW // 2

            anchors = {"ACT": None, "DVE": None, "PE": None}

            def anchored(eng, op):
                a = anchors[eng]
                if a is not None:
                    add_dep_helper(op.ins, a.ins, sync=False,
                                   reason="fill inter-step gap only")
                return op

            def ts_split(dst, srcp, bias, relu, base=0):
                # anchored after the scan chain tail, so these only occupy
                # the inter-step gap in the DVE stream
                d = dst[:, base:base + GW]
                if relu:
                    anchored("DVE", nc.vector.tensor_scalar(
                        d, srcp[:], bias, 0.0, op0=OP.add, op1=OP.max))
                else:
                    anchored("DVE", nc.vector.tensor_scalar(
                        d, srcp[:], bias, None, op0=OP.add))

            def meas_stage(s, gg, phic_of, yc_of):
                ci, g = divmod(gg, NG)
                cs = slice(g * GW, (g + 1) * GW)
                st = meas_state.setdefault(gg, {})
                if s == 0:
                    psA = psm_p.tile([HD, GW], F32, tag="pm", name="pm")
                    anchored("PE", mm(psA[:], W["dw0aT"][:], phic_of(ci)[:, cs], start=True, stop=False))
                    mm(psA[:], W["dw0bT"][:], h_hist[:, cs], start=False, stop=True)
                    st["dh"] = meas.tile([HD, GW], BF, tag="dh", name="dh", bufs=4)
                    anchored("ACT", act(st["dh"][:], psA[:], AF.Relu,
                                        bias=BI["db0"][:]))
                elif s == 1:
                    psD = psm_p.tile([HD, GW], F32, tag="pm", name="pm")
                    anchored("PE", mm(psD[:], W["fwT"][:], st["dh"][:], start=True, stop=True))
                    st["ph"] = meas.tile([HD, GW], BF, tag="ph", name="ph", bufs=3)
                    anchored("ACT", act(st["ph"][:], psD[:], AF.Relu,
                                        bias=BI["fb"][:]))
                elif s == 2:
                    psF = psm_p.tile([HD, GW], F32, tag="pm", name="pm")
                    anchored("PE", mm(psF[:], W["gT"][:], st.pop("ph")[:], start=True, stop=True))
                    st["mh"] = meas.tile([HD, GW], BF, tag="mh", name="mh", bufs=3)
                    anchored("ACT", act(st["mh"][:], psF[:], AF.Relu,
                                        bias=BI["gb"][:]))
                elif s == 3:
                    psY = psm_p.tile([YD, GW], F32, tag="pm", name="pm")
                    anchored("PE", mm(psY[:], W["mw1T"][:], st.pop("mh")[:], start=True, stop=False))
                    mm(psY[:], W["cxwT"][:], st.pop("dh")[:], start=False, stop=True)
                    jA = meas.tile([YD, GW], F32, tag="jA", name="jA", bufs=3)
                    anchored("ACT", act(jA[:], psY[:], AF.Square,
                                        bias=BI["mb1"][:],
                                        accum_out=sq_acc[:, gg:gg + 1]))
                    jB = meas.tile([YD, GW], F32, tag="jB", name="jB", bufs=3)
                    anchored("DVE", nc.vector.affine_mul_reduce(
                        jB[:], xy_acc[:, gg:gg + 1],
                        psY[:], yc_of(ci)[:, cs], 1.0, BI["mb1"][:]))
                    del meas_state[gg]

            # Bulk: hid = relu(pw0 @ u + pb0).  phi_u's second linear layer
            # is host-fused into wih0/dw0a, so `hid` IS the scan/meas input.
            def bulk_stage(s, bb, uc_of, phid_of):
                ci, g = divmod(bb, NG)
                cs = slice(g * GW, (g + 1) * GW)
                ps1 = psb_p.tile([HD, GW], F32, tag="pb", name="pb")
                anchored("PE", mm(ps1[:], W["pw0T"][:], uc_of(ci)[:, cs], start=True, stop=True))
                ts_split(phid_of(ci), ps1, BI["pb0"][:], True, base=g * GW)

            # ---- main schedule ----
            for _rep in range(reps):
                uy = {0: load_uy(0)}
                phic_of = lambda ci: phis[ci % 2]
                uc_of = lambda ci: uy[ci][0]
                yc_of = lambda ci: uy[ci][1]
                # prologue: chunk-0 hid
                for g in range(NG):
                    bulk_stage(0, g, uc_of, phic_of)
                if NCH > 1:
                    uy[1] = load_uy(1)
                for u in range(T_total + 10):
                    emit_scan_step(u, phic_of)
                    # measurement: stage s of group gg at step 4*gg+5+s
                    # (h1(t) lands at phase t+2 now that L1 lags two phases)
                    for s in range(N_MEAS_STAGES):
                        r = u - 5 - s
                        if r >= 0 and r % 4 == 0:
                            gg = r // 4
                            if gg < NQ:
                                meas_stage(s, gg, phic_of, yc_of)
                    # bulk for the next chunk: one stage per quad, placed on
                    # the u%4==1 phases (measured fastest there)
                    r = u - 1
                    if r >= 0 and r % 4 == 0:
                        bb = r // 4 + NG
                        if NG <= bb < NQ:
                            bulk_stage(0, bb, uc_of, phic_of)
                    # io prefetch / release bookkeeping at chunk boundaries
                    if u < T_total and u % Tc == Tc - 1:
                        ci = u // Tc
                        if ci + 2 < NCH:
                            uy[ci + 2] = load_uy(ci + 2)
                        uy.pop(ci - 1, None)
            dma(out_p[:, 0:NGT], sq_acc[:])
            dma(out_p[:, NGT:2 * NGT], xy_acc[:])

    nc.compile()
    meta = dict(T_total=T_total, Tc=Tc, NGT=NGT)
    return nc, meta


def prep_inputs(inputs, T_total=T):
    """Host-side shard + relayout.  Returns (in_maps, y_sq_sum)."""
    u = np.asarray(inputs["u"], np.float32)
    y = np.asarray(inputs["y"], np.float32)
    h0 = np.asarray(inputs["h0"], np.float32)
    if T_total != T:
        u = u[:, :, :T_total]
        y = y[:, :, :T_total]

    g = lambda n: np.asarray(inputs[n], np.float64)
    wih, whh = g("gru_wih"), g("gru_whh")
    # host-fused linear-linear boundaries of the measurement branch:
    #   xmv = Xc @ (W1 dh + b1) + xb  ->  XW1 dh + xb2
    #   ph  = relu(Px0 xmv' + pb0)    where xmv' is the same affine of dh
    #   mh  = relu(Mw0 (Px1 ph + pb1) + mb0) -> G ph + gb
    Xc = np.concatenate([g("x_mean_w"), g("x_logvar_w")], 0)   # (32,128)
    xb = np.concatenate([g("x_mean_b"), g("x_logvar_b")], 0)
    XW1 = Xc @ g("dynn_w1")
    xb2 = Xc @ g("dynn_b1") + xb
    FW = g("phi_x_w0") @ XW1
    fb = g("phi_x_w0") @ xb2 + g("phi_x_b0")
    G = g("menn_w0") @ g("phi_x_w1")
    gb = g("menn_w0") @ g("phi_x_b1") + g("menn_b0")
    # physics term C @ x_mean host-fused straight onto dh:
    #   C @ x_mean = (C @ XW1[:ZD]) @ dh + C @ xb2[:ZD]
    CXW = g("C") @ XW1[:ZD]                  # (YD, HD)
    mb2 = g("menn_b1") + g("C") @ xb2[:ZD]   # (YD,)
    # fuse phi_u's second linear layer into all its (linear) consumers:
    #   wih0 @ phi_u = (wih0 @ W1) @ hid + wih0 @ b1
    #   dynn_a @ phi_u = (dynn_a @ W1) @ hid + dynn_a @ b1
    W1, b1 = g("phi_u_w1"), g("phi_u_b1")
    gru_b = wih[0] @ b1                       # (384,) layer-0 gate bias
    ind3 = np.zeros((3, 3 * HD))
    for k in range(3):
        ind3[k, k * HD:(k + 1) * HD] = 1.0
    shared = {
        "wih0T": (wih[0] @ W1).T, "whh0T": whh[0].T,
        "wih1T": wih[1].T, "whh1T": whh[1].T,
        "pw0T": g("phi_u_w0").T,
        "b3": gru_b.reshape(3, HD), "ind3": ind3,
        "dw0aT": (g("dynn_w0")[:, :HD] @ W1).T, "dw0bT": g("dynn_w0")[:, HD:].T,
        "fwT": FW.T, "gT": G.T,
        "mw1T": g("menn_w1").T, "cxwT": CXW.T,
        "ident": np.eye(HD, dtype=np.float64),
    }
    shared = {k: np.ascontiguousarray(v, dtype=BF_NP) for k, v in shared.items()}
    biases = {
        "pb0": g("phi_u_b0"),
        "db0": g("dynn_b0") + g("dynn_w0")[:, :HD] @ b1,
        "fb": fb, "gb": gb,
        "mb1": mb2,
    }
    for k, v in biases.items():
        shared[k] = np.ascontiguousarray(v[:, None], dtype=np.float32)

    in_maps = []
    for c in range(NCORES):
        bs = slice(c * BL, (c + 1) * BL)
        ub = u[bs]                      # (BL, UD, Tt)
        yb = y[bs]
        m = dict(shared)
        m["u"] = np.ascontiguousarray(
            ub.transpose(1, 2, 0).reshape(UD, -1), dtype=BF_NP)
        m["y"] = np.ascontiguousarray(
            yb.transpose(1, 2, 0).reshape(YD, -1), dtype=BF_NP)
        h0b = h0[:, bs, :]              # (L, BL, HD)
        m["h0f"] = np.ascontiguousarray(
            np.concatenate([h0b[0].T, h0b[1].T], 1), dtype=BF_NP)
        in_maps.append(m)

    y_sq = float(np.dot(y.reshape(-1).astype(np.float64),
                        y.reshape(-1).astype(np.float64)))
    return in_maps, y_sq


def reduce_outputs(results, meta, y_sq):
    NGT = meta["NGT"]
    total = 0.0
    for r in results:
        o = np.asarray(r["out"], np.float64)
        total += o[:, :NGT].sum() - 2.0 * o[:, NGT:].sum()
    return np.float32(total + y_sq)


_CACHE = {}


def kernel(**inputs):
    key = ("full", T, 64)
    if key not in _CACHE:
        _CACHE[key] = build(T, 64)
    nc, meta = _CACHE[key]
    in_maps, y_sq = prep_inputs(inputs, T)
    res = run_bass_kernel_spmd(nc, in_maps, core_ids=list(range(NCORES)))
    return reduce_outputs(res.results, meta, y_sq)

